# revision 2
# baseline (speedup 1.0000x reference)
"""Additive (Bahdanau) attention kernel for Trainium2, 8 NeuronCores.

Problem: B=4, H=16, L=8192, D=64 (fp32)
    e1 = q @ Wa_w.T + Wa_b ; e2 = k @ Ua_w.T + Ua_b
    t  = tanh(e1 + e2)
    e  = t @ va_w[0] + va_b          (va_b dropped: softmax shift-invariant)
    e  = where(mask == 0, -1e4, e)   (additive -1e4 bias before exp)
    alpha = softmax(e, axis=-1)      (over L)
    out = alpha[..., None] * v

Sharding: 64 independent (b, h) slices -> 8 per core, no collectives.

Per-slice layout: l = p * (L/128) + j with p = SBUF partition, j = tile
column; q/k/v live as [128, J, 64]; q/k/v are cast to bf16 by the SWDGE
DMA.  j-tiles processed in PAIRS (TensorE transpose), pairs in GROUPS of
4 (batched block-diag weight matmuls N=512 + tanh), per-pair score
matmul with block-va -> scores in natural [128, 1] columns.

DMA schedule (the kernel is HBM-bound at ~26 GB/s x 16 SDMA engines):
- consts ride ONE stride-forced HWDGE blob (128 per-partition
  descriptors spread over all 16 engines; a coalesced load lands on a
  single engine lane and gives that lane a multi-us backlog that
  surfaces as a dead-bandwidth bubble at the END of the stream).
- SWDGE (gpsimd) cannot emit before ~8.7us (engine-start barrier + Q7
  ring init), so slice 0's first half is staged as f32 via HWDGE from
  ~2.5us and cast-copied to bf16 on the otherwise-idle ScalarE/DVE.
- maskb is loaded by SWDGE (always partition-sprayed), not HWDGE.
- the LAST slice loads v in 4 quarter-DMAs *after* its q/k taper so the
  final output muls chase the v arrivals; softmax uses per-quarter exp
  with partial rowsums, unnormalized o_raw = p (x) v on DVE, and the
  final 1/sum scale split across ScalarE (per-partition scale) and DVE.

Softmax: additive -1e4 mask bias (DVE), exp with fused per-row
accumulate (ScalarE accum_out) into bf16 p_sb, cross-partition sum via
ones-matmul on TensorE, reciprocal on DVE, broadcast multiply with v.
Slices 0..6 defer softmax/v-scale/store into the next slice's group
loop (stages at g1..g6) so no engine blocks in-line.
"""

import numpy as np
import ml_dtypes
from contextlib import ExitStack

import concourse.bass as bass
import concourse.tile as tile
from concourse import bacc, mybir
from concourse.bass_utils import run_bass_kernel_spmd

B, H, L, D = 4, 16, 8192, 64
N_CORES = 8
SLICES_PER_CORE = (B * H) // N_CORES
P = 128

F32 = mybir.dt.float32
BF16 = mybir.dt.bfloat16
I32 = mybir.dt.int32

CB_COLS = 4 * P + 3          # wblk | ublk | ident | ones | vablk(2) | bias2(1)
CB_PAD = 576                 # padded row so the HWDGE load cannot coalesce


def build_bass(n_slices=SLICES_PER_CORE, seq=L, pipeline=True):
    JT = seq // P            # j-columns per slice
    NPAIR = JT // 2          # tile pairs per slice
    NGRP = NPAIR // 4        # groups of 4 pairs
    QJ = JT // 4             # j-columns per softmax quarter (last slice)
    assert NPAIR % 4 == 0 and JT % 8 == 0

    nc = bacc.Bacc(target_bir_lowering=False)
    q_ext = nc.declare_dram_parameter("q", [n_slices, seq, D], F32, isOutput=False)
    k_ext = nc.declare_dram_parameter("k", [n_slices, seq, D], F32, isOutput=False)
    v_ext = nc.declare_dram_parameter("v", [n_slices, seq, D], F32, isOutput=False)
    # additive mask bias, precomputed on host in on-chip layout [p, s, j]
    mb_ext = nc.declare_dram_parameter("maskb", [P, n_slices * (seq // P)], F32,
                                       isOutput=False)
    cb_ext = nc.declare_dram_parameter("cblk", [P, CB_PAD], BF16, isOutput=False)
    out_ext = nc.declare_dram_parameter("out", [n_slices, seq, D], BF16,
                                        isOutput=True)

    def qr(ext, s):
        return ext[s].rearrange("(p j) d -> p j d", p=P)

    with tile.TileContext(nc) as tc, ExitStack() as ctx:
        consts = ctx.enter_context(tc.tile_pool(name="consts", bufs=1))
        stage = ctx.enter_context(tc.tile_pool(name="stage", bufs=1))
        bigq = ctx.enter_context(tc.tile_pool(name="bigq", bufs=2))
        bigk = ctx.enter_context(tc.tile_pool(name="bigk", bufs=2))
        bigv = ctx.enter_context(tc.tile_pool(name="bigv", bufs=3))
        bigo = ctx.enter_context(tc.tile_pool(name="bigo", bufs=2))
        chunks = ctx.enter_context(tc.tile_pool(name="chunks", bufs=3))
        smalls = ctx.enter_context(tc.tile_pool(name="smalls", bufs=2))
        ps_t = ctx.enter_context(tc.tile_pool(name="ps_t", bufs=3, space="PSUM"))
        ps_e = ctx.enter_context(tc.tile_pool(name="ps_e", bufs=3, space="PSUM"))
        ps_sc = ctx.enter_context(tc.tile_pool(name="ps_sc", bufs=2, space="PSUM"))

        # ---- HWDGE (live from ~2.5us): consts blob + f32 staging of the
        # first half of slice 0 (SWDGE data cannot arrive before ~9us)
        cb = consts.tile([P, CB_COLS], BF16)
        nc.sync.dma_start(cb[:], cb_ext[:, 0:CB_COLS])
        wblk = cb[:, 0:P]
        ublk = cb[:, P:2 * P]
        ident = cb[:, 2 * P:3 * P]
        ones = cb[:, 3 * P:4 * P]
        vablk = cb[:, 4 * P:4 * P + 2]
        bias2 = cb[:, 4 * P + 2:4 * P + 3]

        jq8 = JT // 8
        stq = stage.tile([P, JT // 2, D], F32)
        stk = stage.tile([P, JT // 2, D], F32)
        for c in range(4):
            js = slice(c * jq8, (c + 1) * jq8)
            nc.sync.dma_start(stq[:, js, :], qr(q_ext, 0)[:, js, :])
            nc.sync.dma_start(stk[:, js, :], qr(k_ext, 0)[:, js, :])

        maskb_all = consts.tile([P, n_slices, JT], F32)

        mid_splits = [(0, JT // 2), (JT // 2, JT)]
        last_splits = [(0, JT // 2), (JT // 2, 3 * JT // 4),
                       (3 * JT // 4, 7 * JT // 8), (7 * JT // 8, JT)]

        pending = None
        for s in range(n_slices):
            last = s == n_slices - 1
            q_sb = bigq.tile([P, JT, D], BF16, name="q_sb", tag="q_sb")
            k_sb = bigk.tile([P, JT, D], BF16, name="k_sb", tag="k_sb")
            v_sb = bigv.tile([P, JT, D], BF16, name="v_sb", tag="v_sb")
            if s == 0:
                # SWDGE: second half only; first half comes from the stage
                for j0, j1 in [(JT // 2, 3 * JT // 4), (3 * JT // 4, JT)]:
                    js = slice(j0, j1)
                    nc.gpsimd.dma_start(q_sb[:, js, :], qr(q_ext, s)[:, js, :])
                    nc.gpsimd.dma_start(k_sb[:, js, :], qr(k_ext, s)[:, js, :])
                # maskb sprays across all 16 engines on the SWDGE path
                nc.gpsimd.dma_start(
                    maskb_all[:],
                    mb_ext.rearrange("p (s j) -> p s j", s=n_slices))
                nc.gpsimd.dma_start(v_sb[:], qr(v_ext, s))
                # cast-copies stage(f32) -> q_sb/k_sb(bf16), engines alternate
                for c in range(4):
                    js = slice(c * jq8, (c + 1) * jq8)
                    if c % 2 == 0:
                        nc.vector.tensor_copy(q_sb[:, js, :], stq[:, js, :])
                        nc.scalar.copy(k_sb[:, js, :], stk[:, js, :])
                    else:
                        nc.scalar.copy(q_sb[:, js, :], stq[:, js, :])
                        nc.vector.tensor_copy(k_sb[:, js, :], stk[:, js, :])
            elif not last:
                for j0, j1 in mid_splits:
                    js = slice(j0, j1)
                    nc.gpsimd.dma_start(q_sb[:, js, :], qr(q_ext, s)[:, js, :])
                    nc.gpsimd.dma_start(k_sb[:, js, :], qr(k_ext, s)[:, js, :])
                nc.gpsimd.dma_start(v_sb[:], qr(v_ext, s))
            else:
                # last slice: q/k taper first, then v in QUARTER DMAs so the
                # final output muls can chase the v arrivals
                for j0, j1 in last_splits:
                    js = slice(j0, j1)
                    nc.gpsimd.dma_start(q_sb[:, js, :], qr(q_ext, s)[:, js, :])
                    nc.gpsimd.dma_start(k_sb[:, js, :], qr(k_ext, s)[:, js, :])
                for qi in range(4):
                    js = slice(qi * QJ, (qi + 1) * QJ)
                    nc.gpsimd.dma_start(v_sb[:, js, :], qr(v_ext, s)[:, js, :])

            scores_ps = ps_sc.tile([P, JT], F32)

            def make_state():
                return {
                    "s": s, "scores_ps": scores_ps, "v_sb": v_sb,
                    "sm": smalls.tile([P, JT], F32, tag="sm", name="sm"),
                    "p_sb": smalls.tile([P, JT], BF16, tag="p_sb", name="p_sb"),
                    "rowsum": smalls.tile([P, 1], F32, tag="rowsum",
                                          name="rowsum"),
                    "rowsum_bf": smalls.tile([P, 1], BF16, tag="rowsum_bf",
                                             name="rowsum_bf"),
                    "invs": smalls.tile([P, 1], F32, tag="invs", name="invs"),
                    "alpha": smalls.tile([P, JT], BF16, tag="alpha",
                                         name="alpha"),
                    "o_sb": bigo.tile([P, JT, D], BF16, tag="o_sb",
                                      name="o_sb"),
                }

            def emit_softmax_stage(st, stage_i):
                """Deferred softmax/v-scale/store for a previous slice."""
                if stage_i == 0:
                    nc.vector.tensor_add(st["sm"][:], st["scores_ps"][:],
                                         maskb_all[:, st["s"], :])
                    nc.scalar.activation(st["p_sb"][:], st["sm"][:],
                                         mybir.ActivationFunctionType.Exp,
                                         accum_out=st["rowsum"][:])
                elif stage_i == 1:
                    nc.vector.tensor_copy(st["rowsum_bf"][:], st["rowsum"][:])
                    nc.tensor.matmul(st["scores_ps"][:, 0:1], ones[:],
                                     st["rowsum_bf"][:], start=True, stop=True)
                elif stage_i == 2:
                    nc.vector.reciprocal(st["invs"][:], st["scores_ps"][:, 0:1])
                    nc.vector.tensor_scalar_mul(st["alpha"][:], st["p_sb"][:],
                                                st["invs"][:])
                elif stage_i in (3, 4):
                    h0 = (stage_i - 3) * (JT // 2)
                    jh = slice(h0, h0 + JT // 2)
                    nc.vector.tensor_mul(
                        st["o_sb"][:, jh, :], st["v_sb"][:, jh, :],
                        st["alpha"][:, jh, None].to_broadcast([P, JT // 2, D]))
                    nc.sync.dma_start(qr(out_ext, st["s"])[:, jh, :],
                                      st["o_sb"][:, jh, :])

            if last:
                cur = make_state()
                cur["rs_q"] = [smalls.tile([P, 1], F32, tag=f"rs_q{i}",
                                           name=f"rs_q{i}") for i in range(4)]
            else:
                cur = None

            if pipeline:
                stage_pos = [min(1, NGRP - 1), min(2, NGRP - 1),
                             min(3, NGRP - 1), min(4, NGRP - 1),
                             min(6, NGRP - 1)]
            else:
                stage_pos = [-1] * 5
            for g in range(NGRP):
                if pending is not None:
                    for stg in range(5):
                        if stage_pos[stg] == g:
                            emit_softmax_stage(pending, stg)
                # [128, pair, 256]: per pair cols 0:128 = qT2, 128:256 = kT2
                qkT4 = chunks.tile([P, 4, 2 * P], BF16, tag="qkT4")
                for h in range(2):          # two 2-pair transpose blocks
                    pQK2 = ps_t.tile([P, 4 * P], BF16)
                    for b in range(2):      # pair within block
                        u = g * 4 + 2 * h + b
                        j0 = 2 * u
                        nc.tensor.transpose(
                            pQK2[:, 2 * b * P:(2 * b + 1) * P],
                            q_sb[:, j0:j0 + 2, :].rearrange("p a d -> p (a d)"),
                            ident[:])
                        nc.tensor.transpose(
                            pQK2[:, (2 * b + 1) * P:(2 * b + 2) * P],
                            k_sb[:, j0:j0 + 2, :].rearrange("p a d -> p (a d)"),
                            ident[:])
                    dst = qkT4[:, 2 * h:2 * h + 2, :].rearrange("p a c -> p (a c)")
                    if h == 0:
                        nc.vector.tensor_copy(dst, pQK2[:])
                    else:
                        nc.scalar.copy(dst, pQK2[:])
                pE4 = ps_e.tile([P, 4 * P], F32)
                nc.tensor.matmul(pE4.rearrange("p (a c) -> p a c", a=4),
                                 wblk[:], qkT4[:, :, 0:P],
                                 start=True, stop=False)
                nc.tensor.matmul(pE4.rearrange("p (a c) -> p a c", a=4),
                                 ublk[:], qkT4[:, :, P:2 * P],
                                 start=False, stop=True)
                tT4 = chunks.tile([P, 4 * P], BF16, tag="tT4")
                nc.scalar.activation(tT4[:], pE4[:],
                                     mybir.ActivationFunctionType.Tanh,
                                     bias=bias2[:], scale=1.0)
                for pr in range(4):
                    j0 = 2 * (g * 4 + pr)
                    nc.tensor.matmul(scores_ps[:, j0:j0 + 2],
                                     tT4[:, pr * P:(pr + 1) * P], vablk[:],
                                     start=True, stop=True)
                if cur is not None and g % 2 == 1 and g < NGRP - 1:
                    # last slice: per-quarter masked exp as soon as the
                    # quarter's scores exist (quarter 3 runs after the loop)
                    qi = g // 2
                    qs = slice(qi * QJ, (qi + 1) * QJ)
                    nc.vector.tensor_add(cur["sm"][:, qs], scores_ps[:, qs],
                                         maskb_all[:, s, qs])
                    nc.scalar.activation(cur["p_sb"][:, qs], cur["sm"][:, qs],
                                         mybir.ActivationFunctionType.Exp,
                                         accum_out=cur["rs_q"][qi][:])

            if last:
                pend6 = pending
                pending = cur
            else:
                pending = cur if cur is not None else make_state()
            if not pipeline and not last:
                for stage_i in range(5):
                    emit_softmax_stage(pending, stage_i)
                pending = None

        # ---- final slice drain: quarter-3 exp, rowsum combine on ScalarE,
        # o_raw = p (x) v on DVE chasing the v quarter DMAs, final
        # per-partition 1/sum scale split across ScalarE and DVE, stores
        # chasing per quarter.
        st = pending
        Exp = mybir.ActivationFunctionType.Exp
        q3 = slice(3 * QJ, 4 * QJ)
        nc.vector.tensor_add(st["sm"][:, q3], st["scores_ps"][:, q3],
                             maskb_all[:, st["s"], q3])
        nc.scalar.activation(st["p_sb"][:, q3], st["sm"][:, q3], Exp,
                             accum_out=st["rs_q"][3][:])
        # rowsum = sum of quarter partials (ScalarE Identity-with-bias adds)
        nc.scalar.add(st["rowsum"][:], st["rs_q"][0][:], st["rs_q"][1][:])
        nc.scalar.add(st["rowsum"][:], st["rowsum"][:], st["rs_q"][2][:])
        nc.scalar.add(st["rowsum"][:], st["rowsum"][:], st["rs_q"][3][:])
        nc.scalar.copy(st["rowsum_bf"][:], st["rowsum"][:])
        nc.tensor.matmul(st["scores_ps"][:, 0:1], ones[:], st["rowsum_bf"][:],
                         start=True, stop=True)
        o_raw = st["o_sb"]
        o_fin = bigo.tile([P, JT, D], BF16, tag="o_sb", name="o_fin")
        # DVE: o_raw quarters 0,1 -> reciprocal -> o_raw quarter 2 -> alpha3
        for qi in range(2):
            qs = slice(qi * QJ, (qi + 1) * QJ)
            nc.vector.tensor_mul(
                o_raw[:, qs, :], st["v_sb"][:, qs, :],
                st["p_sb"][:, qs, None].to_broadcast([P, QJ, D]))
        nc.vector.reciprocal(st["invs"][:], st["scores_ps"][:, 0:1])
        qs2 = slice(2 * QJ, 3 * QJ)
        nc.vector.tensor_mul(
            o_raw[:, qs2, :], st["v_sb"][:, qs2, :],
            st["p_sb"][:, qs2, None].to_broadcast([P, QJ, D]))
        # ScalarE: final scale of quarters 0..2 (per-partition scalar mul)
        for qi in range(3):
            qs = slice(qi * QJ, (qi + 1) * QJ)
            nc.scalar.mul(o_fin[:, qs, :], o_raw[:, qs, :], st["invs"][:])
            nc.sync.dma_start(qr(out_ext, st["s"])[:, qs, :], o_fin[:, qs, :])
        # quarter 3 via alpha path on DVE, split in two to chase v arrival
        nc.vector.tensor_scalar_mul(st["alpha"][:, q3], st["p_sb"][:, q3],
                                    st["invs"][:])
        for hi in range(2):
            js = slice(3 * QJ + hi * (QJ // 2), 3 * QJ + (hi + 1) * (QJ // 2))
            nc.vector.tensor_mul(
                o_fin[:, js, :], st["v_sb"][:, js, :],
                st["alpha"][:, js, None].to_broadcast([P, QJ // 2, D]))
            nc.sync.dma_start(qr(out_ext, st["s"])[:, js, :], o_fin[:, js, :])

    nc.compile()
    return nc


def make_host_inputs(q, k, v, mask, Wa_w, Wa_b, Ua_w, Ua_b, va_w):
    """Returns per-core in_maps for the full problem."""
    q = np.ascontiguousarray(np.asarray(q, np.float32).reshape(B * H, L, D))
    k = np.ascontiguousarray(np.asarray(k, np.float32).reshape(B * H, L, D))
    v = np.ascontiguousarray(np.asarray(v, np.float32).reshape(B * H, L, D))
    JT = L // P
    mask = np.asarray(mask, np.int32).reshape(B * H, P, JT)
    maskb = np.where(mask == 0, np.float32(-10000.0), np.float32(0.0))

    bf16 = ml_dtypes.bfloat16
    WaT = np.asarray(Wa_w, np.float32).T  # [d, e]
    UaT = np.asarray(Ua_w, np.float32).T
    wblk = np.zeros((P, P), np.float32)
    wblk[0:D, 0:D] = WaT
    wblk[D:2 * D, D:2 * D] = WaT
    ublk = np.zeros((P, P), np.float32)
    ublk[0:D, 0:D] = UaT
    ublk[D:2 * D, D:2 * D] = UaT
    be = (np.asarray(Wa_b, np.float32) + np.asarray(Ua_b, np.float32))
    bias2 = np.concatenate([be, be]).reshape(P, 1)
    va = np.asarray(va_w, np.float32)[0]
    vablk = np.zeros((P, 2), np.float32)
    vablk[0:D, 0] = va
    vablk[D:2 * D, 1] = va
    ident = np.eye(P, dtype=np.float32)
    ones = np.ones((P, P), dtype=np.float32)

    cblk = np.zeros((P, CB_PAD), np.float32)
    cblk[:, 0:P] = wblk
    cblk[:, P:2 * P] = ublk
    cblk[:, 2 * P:3 * P] = ident
    cblk[:, 3 * P:4 * P] = ones
    cblk[:, 4 * P:4 * P + 2] = vablk
    cblk[:, 4 * P + 2:4 * P + 3] = bias2
    cblk = cblk.astype(bf16)

    in_maps = []
    for i in range(N_CORES):
        sl = slice(i * SLICES_PER_CORE, (i + 1) * SLICES_PER_CORE)
        # [s, p, j] -> [p, s*j] so the device load is contiguous per partition
        mb = np.ascontiguousarray(
            maskb[sl].transpose(1, 0, 2).reshape(P, SLICES_PER_CORE * JT))
        in_maps.append({
            "q": q[sl], "k": k[sl], "v": v[sl], "maskb": mb, "cblk": cblk,
        })
    return in_maps


_CACHED_NC = None


def kernel(q, k, v, mask, Wa_w, Wa_b, Ua_w, Ua_b, va_w, va_b=None, **kwargs):
    global _CACHED_NC
    if _CACHED_NC is None:
        _CACHED_NC = build_bass()
    in_maps = make_host_inputs(q, k, v, mask, Wa_w, Wa_b, Ua_w, Ua_b, va_w)
    res = run_bass_kernel_spmd(_CACHED_NC, in_maps, list(range(N_CORES)))
    out = np.concatenate([np.asarray(r["out"], np.float32) for r in res.results],
                         axis=0)
    return np.ascontiguousarray(out.reshape(B, H, L, D).astype(np.float32))
